# revision 9
# baseline (speedup 1.0000x reference)
"""Trainium2 Bass kernel for nn_CPCircuitLayer.

Math: with all_indices the full cartesian grid (s = n // H, h = n % H),
    out[b, s, h] = sum_r seq_emb[b,s,r] * hid_emb[b,h,r] * cp[r]
                 = (seq_emb[b] @ diag(cp) @ hid_emb[b].T)[s, h]
where seq_emb[b] = X_b @ seq_W.T  (X_b = hidden_states[b], contract H)
      hid_emb[b] = X_b.T @ hid_W.T                        (contract S)

Sharding: 8 cores = (batch b, seq half) pairs. Each core holds X_b fully
(the hid factor contracts over all of S) with rows rotated so its own
seq half comes first; it PE-transposes those first 512 rows for the seq
factor, computes
    hid_embT = (hid_W*cp) @ X_b          [R, H]
    seq_embT = seq_W @ X_b[half].T       [R, S/2]
    out_half = seq_embT.T @ hid_embT     [S/2, H]
and writes its [512, 1024] slice of the output.

Matmuls run in FP32R (fp32 rounded-to-nearest at 12 mantissa bits; the
PE streams it at full rate). Inputs are pre-rounded to the FP32R bit
format on the host so the device does no conversion work. A few dummy
matmuls at kernel start warm the PE HAM clock gate before real work.
"""

import numpy as np

B, S, H, R = 4, 1024, 1024, 32
N_CORES = 8
SH = S // 2   # seq rows per core
KT = S // 128  # k-tiles over the contraction dims
MT = SH // 128  # row tiles in this core's seq half

_compiled = {}


def _np_fallback(hidden_states, all_indices, seq_W, hid_W, cp_weight):
    seq_emb = np.einsum("bsh,rh->bsr", hidden_states, seq_W)
    hid_emb = np.einsum("bsh,rs->bhr", hidden_states, hid_W)
    s_idx = all_indices[:, 0].astype(np.int64)
    h_idx = all_indices[:, 1].astype(np.int64)
    g_seq = seq_emb[:, s_idx, :]
    g_hid = hid_emb[:, h_idx, :]
    out = np.einsum("bnr,bnr,r->bn", g_seq, g_hid, cp_weight[0])
    return out.reshape(B, S, H).astype(np.float32)


def _round_f32r(a):
    """Round fp32 to the FP32R format (RNE at 12 mantissa bits), bit-exact
    with the device's own fp32->fp32r conversion."""
    b = np.ascontiguousarray(a, dtype=np.float32).view(np.uint32)
    r = (b + np.uint32(0x7FF) + ((b >> np.uint32(12)) & np.uint32(1))) \
        & np.uint32(0xFFFFF000)
    return r.view(np.float32)


def _wtile(w):
    """[K, R] -> [128, KT*R] tile layout, partition-contiguous."""
    return np.ascontiguousarray(
        w.reshape(KT, 128, R).transpose(1, 0, 2).reshape(128, KT * R))


def _build_program():
    import concourse.mybir as mybir
    import concourse.tile as tile
    from concourse import bacc
    from concourse.masks import make_identity

    f32 = mybir.dt.float32
    f32r = mybir.dt.float32r

    nc = bacc.Bacc("TRN2", target_bir_lowering=False, debug=False,
                   num_devices=N_CORES, enable_partition_id=False)

    x_d = nc.dram_tensor("x", [S, H], f32r, kind="ExternalInput")
    sw_d = nc.dram_tensor("sw", [128, KT * R], f32r, kind="ExternalInput")
    hw_d = nc.dram_tensor("hw", [128, KT * R], f32r, kind="ExternalInput")
    out_d = nc.dram_tensor("out", [SH, H], f32, kind="ExternalOutput")

    with tile.TileContext(nc) as tc:
        with (
            tc.tile_pool(name="xp", bufs=1) as xp,
            tc.tile_pool(name="wp", bufs=1) as wp,
            tc.tile_pool(name="fp", bufs=1) as fp,
            tc.tile_pool(name="op", bufs=2) as op,
            tc.tile_pool(name="wps", bufs=1, space="PSUM") as wps,
            tc.tile_pool(name="tps", bufs=2, space="PSUM") as tps,
            tc.tile_pool(name="fps", bufs=1, space="PSUM") as fps,
            tc.tile_pool(name="ops", bufs=2, space="PSUM") as ops,
        ):
            # --- PE warm-up: dummy matmuls on a memset tile, no deps ---
            warm_f = wp.tile([128, 512], f32)
            nc.gpsimd.memset(warm_f[:], 0.0)
            warm = wp.tile([128, 512], f32r)
            nc.vector.tensor_copy(warm[:], warm_f[:])
            warm_ps = wps.tile([R, 512], f32, name="warm_ps")
            for _ in range(8):
                nc.tensor.matmul(warm_ps[:], warm[:, :R], warm[:],
                                 start=True, stop=True)

            # identity for PE transposes (f32r via a rounding copy so the
            # verifier accepts it as a matmul operand)
            ident_f = wp.tile([128, 128], f32)
            make_identity(nc, ident_f[:])
            ident = wp.tile([128, 128], f32r)
            nc.vector.tensor_copy(ident[:], ident_f[:])

            # weights (small, gate the first matmuls)
            sw_t = wp.tile([128, KT, R], f32r)
            nc.sync.dma_start(sw_t[:], sw_d.rearrange("p (t r) -> p t r", t=KT))
            hw_t = wp.tile([128, KT, R], f32r)
            nc.sync.dma_start(hw_t[:], hw_d.rearrange("p (t r) -> p t r", t=KT))

            # x row-tiles; rows 0..511 are this core's seq half
            x_t = []
            for k in range(KT):
                xk = xp.tile([128, H], f32r, name=f"x{k}")
                nc.sync.dma_start(xk[:], x_d[k * 128:(k + 1) * 128, :])
                x_t.append(xk)

            # transposed seq half: xt_sb[kh][h_local, s_local]
            xt_sb = [fp.tile([128, SH], f32r, name=f"xt{k}") for k in range(KT)]

            hid_ps = [fps.tile([R, 512], f32, name=f"hid_ps{n}") for n in range(2)]

            # per arriving x tile: hid matmuls (all k) + transposes (k < MT)
            for k in range(KT):
                for n in range(2):
                    nc.tensor.matmul(
                        hid_ps[n][:],
                        hw_t[:, k, :],
                        x_t[k][:, n * 512:(n + 1) * 512],
                        start=(k == 0),
                        stop=(k == KT - 1),
                    )
                if k < MT:
                    for kh in range(KT):
                        tr = tps.tile([128, 128], f32r, name="tr")
                        nc.tensor.transpose(
                            tr[:],
                            x_t[k][:, kh * 128:(kh + 1) * 128],
                            ident[:],
                        )
                        nc.vector.tensor_copy(
                            xt_sb[kh][:, k * 128:(k + 1) * 128],
                            tr[:].bitcast(f32),
                        )

            hid_sb = fp.tile([R, H], f32r)
            for n in range(2):
                nc.vector.tensor_copy(
                    hid_sb[:, n * 512:(n + 1) * 512],
                    hid_ps[n][:])

            # seq_embT[r, s] = sum_h seq_wT[h, r] * xt[h, s]
            seq_ps = fps.tile([R, SH], f32)
            for k in range(KT):
                nc.tensor.matmul(
                    seq_ps[:],
                    sw_t[:, k, :],
                    xt_sb[k][:],
                    start=(k == 0),
                    stop=(k == KT - 1),
                )
            seq_sb = fp.tile([R, SH], f32r)
            nc.vector.tensor_copy(seq_sb[:], seq_ps[:])

            # out[s, h] = sum_r seq_embT[r, s] * hid_embT[r, h]
            for m in range(MT):
                o_sb = op.tile([128, H], f32, name="o_sb")
                for n in range(2):
                    o_ps = ops.tile([128, 512], f32, name="o_ps")
                    nc.tensor.matmul(
                        o_ps[:],
                        seq_sb[:, m * 128:(m + 1) * 128],
                        hid_sb[:, n * 512:(n + 1) * 512],
                    )
                    nc.vector.tensor_copy(o_sb[:, n * 512:(n + 1) * 512], o_ps[:])
                nc.sync.dma_start(out_d[m * 128:(m + 1) * 128, :], o_sb[:])

    nc.compile()
    return nc


def _get_program():
    if "nc" not in _compiled:
        _compiled["nc"] = _build_program()
    return _compiled["nc"]


def _make_in_maps(hidden_states, seq_W, hid_W, cp_weight):
    swT = _round_f32r(_wtile(np.ascontiguousarray(seq_W.T)))
    hwT_rows = np.ascontiguousarray((hid_W * cp_weight[0][:, None]).T)  # [S, R]
    # per-half row rotation: own seq half first (hid contraction over S is
    # order-invariant as long as x rows and hw rows permute together)
    hw_rot = [
        _round_f32r(_wtile(np.concatenate(
            [hwT_rows[half * SH:(half + 1) * SH], hwT_rows[:half * SH],
             hwT_rows[(half + 1) * SH:]], axis=0)))
        for half in range(2)
    ]
    in_maps = []
    for c in range(N_CORES):
        b, half = divmod(c, 2)
        xb = _round_f32r(hidden_states[b])
        if half:
            xb = np.ascontiguousarray(
                np.concatenate([xb[SH:], xb[:SH]], axis=0))
        in_maps.append({
            "x": xb,
            "sw": swT,
            "hw": hw_rot[half],
        })
    return in_maps


def kernel(hidden_states, all_indices, seq_W, hid_W, cp_weight):
    hidden_states = np.asarray(hidden_states, dtype=np.float32)
    seq_W = np.asarray(seq_W, dtype=np.float32)
    hid_W = np.asarray(hid_W, dtype=np.float32)
    cp_weight = np.asarray(cp_weight, dtype=np.float32)
    idx = np.asarray(all_indices)

    # The reference's all_indices is always the full cartesian grid; verify
    # cheaply and fall back to a host path if ever not.
    n = np.arange(S * H, dtype=idx.dtype)
    if idx.shape != (S * H, 2) or not (
        np.array_equal(idx[:, 0], n // H) and np.array_equal(idx[:, 1], n % H)
    ):
        return _np_fallback(hidden_states, idx, seq_W, hid_W, cp_weight)

    from concourse.bass_utils import run_bass_kernel_spmd

    nc = _get_program()
    in_maps = _make_in_maps(hidden_states, seq_W, hid_W, cp_weight)
    res = run_bass_kernel_spmd(nc, in_maps, list(range(N_CORES)))

    out = np.empty((B, S, H), dtype=np.float32)
    for c in range(N_CORES):
        b, half = divmod(c, 2)
        out[b, half * SH:(half + 1) * SH, :] = res.results[c]["out"]
    return out


# revision 14
# speedup vs baseline: 1.0727x; 1.0727x over previous
"""Trainium2 Bass kernel for nn_CPCircuitLayer.

Math: with all_indices the full cartesian grid (s = n // H, h = n % H),
    out[b, s, h] = sum_r seq_emb[b,s,r] * hid_emb[b,h,r] * cp[r]
                 = (seq_emb[b] @ diag(cp) @ hid_emb[b].T)[s, h]
where seq_emb[b] = X_b @ seq_W.T  (X_b = hidden_states[b], contract H)
      hid_emb[b] = X_b.T @ hid_W.T                        (contract S)

Sharding: 8 cores = (batch b, seq half) pairs. Each core holds X_b fully
(the hid factor contracts over all of S) with rows rotated so its own
seq half comes first; it PE-transposes those first 512 rows for the seq
factor, computes
    hid_embT = (hid_W*cp) @ X_b          [R, H]
    seq_embT = seq_W @ X_b[half].T       [R, S/2]
    out_half = seq_embT.T @ hid_embT     [S/2, H]
and writes its [512, 1024] slice of the output.

Matmuls run in FP32R (fp32 rounded-to-nearest at 12 mantissa bits; the
PE streams it at full rate). Inputs are pre-rounded to the FP32R bit
format on the host so the device does no conversion work. A few dummy
matmuls at kernel start warm the PE HAM clock gate before real work.
"""

import numpy as np

B, S, H, R = 4, 1024, 1024, 32
N_CORES = 8
SH = S // 2   # seq rows per core
KT = S // 128  # k-tiles over the contraction dims
MT = SH // 128  # row tiles in this core's seq half

_compiled = {}


def _np_fallback(hidden_states, all_indices, seq_W, hid_W, cp_weight):
    seq_emb = np.einsum("bsh,rh->bsr", hidden_states, seq_W)
    hid_emb = np.einsum("bsh,rs->bhr", hidden_states, hid_W)
    s_idx = all_indices[:, 0].astype(np.int64)
    h_idx = all_indices[:, 1].astype(np.int64)
    g_seq = seq_emb[:, s_idx, :]
    g_hid = hid_emb[:, h_idx, :]
    out = np.einsum("bnr,bnr,r->bn", g_seq, g_hid, cp_weight[0])
    return out.reshape(B, S, H).astype(np.float32)


def _round_f32r(a):
    """Round fp32 to the FP32R format (RNE at 12 mantissa bits), bit-exact
    with the device's own fp32->fp32r conversion."""
    b = np.ascontiguousarray(a, dtype=np.float32).view(np.uint32)
    r = (b + np.uint32(0x7FF) + ((b >> np.uint32(12)) & np.uint32(1))) \
        & np.uint32(0xFFFFF000)
    return r.view(np.float32)


def _wtile(w):
    """[K, R] -> [128, KT*R] tile layout, partition-contiguous."""
    return np.ascontiguousarray(
        w.reshape(KT, 128, R).transpose(1, 0, 2).reshape(128, KT * R))


def build_raw_program():
    import contextlib

    import concourse.bass as bass
    import concourse.mybir as mybir

    f32 = mybir.dt.float32
    f32r = mybir.dt.float32r

    nc = bass.Bass("TRN2", target_bir_lowering=False, debug=False,
                   num_devices=N_CORES, enable_partition_id=False)

    x_d = nc.dram_tensor("x", [S, H], f32r, kind="ExternalInput")
    w_d = nc.dram_tensor("w", [128, 2 * KT * R], f32r, kind="ExternalInput")
    out_d = nc.dram_tensor("out", [SH, H], f32, kind="ExternalOutput")

    with contextlib.ExitStack() as _xs:
        E = _xs.enter_context
        w_t = E(nc.sbuf_tensor([128, 2 * KT * R], f32r))  # [p, sw | hw]
        x_t = E(nc.sbuf_tensor([128, KT, H], f32r))
        xt_t = E(nc.sbuf_tensor([128, KT, SH], f32r))
        ident_f = E(nc.sbuf_tensor([128, 128], f32))
        ident = E(nc.sbuf_tensor([128, 128], f32r))
        hid_sb = E(nc.sbuf_tensor([R, H], f32r))
        seq_sb = E(nc.sbuf_tensor([R, SH], f32r))
        o_sb = E(nc.sbuf_tensor([128, MT, H], f32))
        hid_ps = E(nc.psum_tensor([R, H], f32))        # 2 banks
        seq_ps = E(nc.psum_tensor([R, SH], f32))       # 1 bank
        o_ps_a = E(nc.psum_tensor([128, 512], f32))    # 1 bank
        o_ps_b = E(nc.psum_tensor([128, 512], f32))    # 1 bank
        tr_a = E(nc.psum_tensor([128, 256], f32r))     # 1 bank
        tr_b = E(nc.psum_tensor([128, 256], f32r))     # 1 bank
        dma_sem = E(nc.semaphore("dma_sem"))
        w_sem = E(nc.semaphore("w_sem"))
        pe_sem = E(nc.semaphore("pe_sem"))
        tr_sem = E(nc.semaphore("tr_sem"))
        dve_sem = E(nc.semaphore("dve_sem"))
        act_sem = E(nc.semaphore("act_sem"))
        gp_sem = E(nc.semaphore("gp_sem"))
        x_sem = [E(nc.semaphore(f"x_sem{k}")) for k in range(KT)]
        block = E(nc.Block())
        sw = lambda k: w_t.ap()[:, k * R:(k + 1) * R]
        hw = lambda k: w_t.ap()[:, KT * R + k * R:KT * R + (k + 1) * R]

        # ---- pe_sem count map (real matmuls only, in tensor-engine order) --
        # hid k emitted at slots: k0:(1,2) k1:(3,4) k2:(5,6) k3:(7,8)
        # seq k0-3: 9..12 ; hid k4:(13,14) ; seq k4-7: 15..18
        # hid k5:(19,20) k6:(21,22) k7:(23,24) ; final: 25..32
        PE_HID_N0_DONE = 23
        PE_HID_N1_DONE = 24
        PE_SEQ_DONE = 18
        # dve order: 1 ident, 2..17 tr copies, 18 seq_c, 19 hid_c0, 20 hid_c1,
        #            21+ even out copies
        DVE_IDENT = 1
        DVE_TR0 = 1          # tr copy j (1-based) is dve op DVE_TR0 + j
        DVE_SEQC = 18
        DVE_HIDC1 = 20

        @block.gpsimd
        def _(gpsimd):
            nc.gpsimd.memset(ident_f.ap(), 0.0).then_inc(gp_sem, 1)
            nc.gpsimd.affine_select(
                out=ident_f.ap(), in_=ident_f.ap(),
                compare_op=mybir.AluOpType.not_equal,
                fill=1.0, base=0, pattern=[[-1, 128]], channel_multiplier=1,
            ).then_inc(gp_sem, 1)

        @block.sync
        def _(sync):
            sync.dma_start(out=w_t.ap(), in_=w_d[:]).then_inc(w_sem, 16)
            for k in range(KT):
                sync.dma_start(
                    out=x_t.ap()[:, k, :],
                    in_=x_d[k * 128:(k + 1) * 128, :],
                ).then_inc(x_sem[k], 16)
            # out DMAs: (m, n) chunk after its PSUM->SBUF copy lands in o_sb
            for j in range(2 * MT):
                m, n = divmod(j, 2)
                if j % 2 == 0:
                    sync.wait_ge(dve_sem, DVE_HIDC1 + 1 + j // 2)
                else:
                    sync.wait_ge(act_sem, (j + 1) // 2)
                sync.dma_start(
                    out=out_d[m * 128:(m + 1) * 128, n * 512:(n + 1) * 512],
                    in_=o_sb.ap()[:, m, n * 512:(n + 1) * 512],
                ).then_inc(dma_sem, 16)
            sync.wait_ge(dma_sem, 16 * 2 * MT)

        @block.tensor
        def _(tensor):
            def hid_mms(k):
                tensor.wait_ge(x_sem[k], 16)
                for n in range(2):
                    nc.tensor.matmul(
                        hid_ps.ap()[:, n * 512:(n + 1) * 512],
                        hw(k), x_t.ap()[:, k, n * 512:(n + 1) * 512],
                        start=(k == 0), stop=(k == KT - 1),
                    ).then_inc(pe_sem, 1)

            def tr_pair(p):
                # transpose x rows [256p, 256p+256) i.e. x tiles 2p, 2p+1
                tensor.wait_ge(x_sem[2 * p], 16)
                tensor.wait_ge(x_sem[2 * p + 1], 16)
                for kh in range(KT):
                    j = p * KT + kh  # pair index 0..15
                    tr = tr_a if j % 2 == 0 else tr_b
                    if j < 2:
                        tensor.wait_ge(dve_sem, DVE_IDENT)  # identity ready
                    else:
                        # WAR: this PSUM bank's previous copy must be done
                        tensor.wait_ge(dve_sem, DVE_TR0 + j - 1)
                    for i in range(2):
                        ins = nc.tensor.transpose(
                            tr.ap()[:, i * 128:(i + 1) * 128],
                            x_t.ap()[:, 2 * p + i, kh * 128:(kh + 1) * 128],
                            ident.ap(),
                        )
                        if i == 1:
                            ins.then_inc(tr_sem, 1)

            def seq_mms(ks):
                for k in ks:
                    # xt_t[:, k, :] complete once both pairs' kh=k copies done
                    tensor.wait_ge(dve_sem, DVE_TR0 + KT + k + 1)
                    nc.tensor.matmul(
                        seq_ps.ap(), sw(k), xt_t.ap()[:, k, :],
                        start=(k == 0), stop=(k == KT - 1),
                    ).then_inc(pe_sem, 1)

            # warm-up (weights tile is the first DMA)
            tensor.wait_ge(w_sem, 16)
            for _ in range(6):
                nc.tensor.matmul(o_ps_a.ap()[0:R, :], w_t.ap()[:, 0:R],
                                 w_t.ap()[:, 0:512], start=True, stop=True)

            hid_mms(0)
            hid_mms(1)
            tr_pair(0)
            hid_mms(2)
            tr_pair(1)
            hid_mms(3)
            seq_mms(range(0, 4))
            hid_mms(4)
            seq_mms(range(4, KT))
            hid_mms(5)
            hid_mms(6)
            hid_mms(7)

            # final: out[m, n-half] = seq_sbT[m-cols] x hid_sb[n-cols]
            tensor.wait_ge(dve_sem, DVE_HIDC1)
            for j in range(2 * MT):
                m, n = divmod(j, 2)
                o_ps = o_ps_a if j % 2 == 0 else o_ps_b
                if j >= 2:
                    # WAR on the recycled PSUM bank
                    if j % 2 == 0:
                        tensor.wait_ge(dve_sem, DVE_HIDC1 + 1 + (j - 2) // 2)
                    else:
                        tensor.wait_ge(act_sem, (j - 1) // 2)
                nc.tensor.matmul(
                    o_ps.ap(),
                    seq_sb.ap()[:, m * 128:(m + 1) * 128],
                    hid_sb.ap()[:, n * 512:(n + 1) * 512],
                    start=True, stop=True,
                ).then_inc(pe_sem, 1)

        @block.vector
        def _(vector):
            vector.wait_ge(gp_sem, 2)
            nc.vector.tensor_copy(ident.ap(), ident_f.ap()).then_inc(dve_sem, 1)
            for j in range(2 * KT):  # 16 transpose-pair copies
                p, kh = divmod(j, KT)
                tr = tr_a if j % 2 == 0 else tr_b
                vector.wait_ge(tr_sem, j + 1)
                nc.vector.tensor_copy(
                    xt_t.ap()[:, kh, 2 * p * 128:(2 * p + 2) * 128],
                    tr.ap()[:].bitcast(f32),
                ).then_inc(dve_sem, 1)
            vector.wait_ge(pe_sem, PE_SEQ_DONE)
            nc.vector.tensor_copy(
                seq_sb.ap(), seq_ps.ap().bitcast(f32)).then_inc(dve_sem, 1)
            vector.wait_ge(pe_sem, PE_HID_N0_DONE)
            nc.vector.tensor_copy(
                hid_sb.ap()[:, 0:512],
                hid_ps.ap()[:, 0:512].bitcast(f32)).then_inc(dve_sem, 1)
            vector.wait_ge(pe_sem, PE_HID_N1_DONE)
            nc.vector.tensor_copy(
                hid_sb.ap()[:, 512:1024],
                hid_ps.ap()[:, 512:1024].bitcast(f32)).then_inc(dve_sem, 1)
            for j in range(0, 2 * MT, 2):  # even out copies
                m, n = divmod(j, 2)
                vector.wait_ge(pe_sem, 24 + j + 1)
                nc.vector.tensor_copy(
                    o_sb.ap()[:, m, n * 512:(n + 1) * 512],
                    o_ps_a.ap(),
                ).then_inc(dve_sem, 1)

        @block.scalar
        def _(scalar):
            for j in range(1, 2 * MT, 2):  # odd out copies
                m, n = divmod(j, 2)
                scalar.wait_ge(pe_sem, 24 + j + 1)
                nc.scalar.copy(
                    o_sb.ap()[:, m, n * 512:(n + 1) * 512],
                    o_ps_b.ap(),
                ).then_inc(act_sem, 1)

    return nc


def _get_program():
    if "nc" not in _compiled:
        _compiled["nc"] = build_raw_program()
    return _compiled["nc"]


def _make_in_maps(hidden_states, seq_W, hid_W, cp_weight):
    swT = _wtile(np.ascontiguousarray(seq_W.T))                    # [128, 256]
    hwT_rows = np.ascontiguousarray((hid_W * cp_weight[0][:, None]).T)  # [S, R]
    # per-half row rotation: own seq half first (hid contraction over S is
    # order-invariant as long as x rows and hw rows permute together)
    w_rot = [
        _round_f32r(np.concatenate([swT, _wtile(np.concatenate(
            [hwT_rows[half * SH:], hwT_rows[:half * SH]], axis=0))], axis=1))
        for half in range(2)
    ]
    in_maps = []
    for c in range(N_CORES):
        b, half = divmod(c, 2)
        xb = _round_f32r(hidden_states[b])
        if half:
            xb = np.ascontiguousarray(
                np.concatenate([xb[SH:], xb[:SH]], axis=0))
        in_maps.append({
            "x": xb,
            "w": w_rot[half],
        })
    return in_maps


def kernel(hidden_states, all_indices, seq_W, hid_W, cp_weight):
    hidden_states = np.asarray(hidden_states, dtype=np.float32)
    seq_W = np.asarray(seq_W, dtype=np.float32)
    hid_W = np.asarray(hid_W, dtype=np.float32)
    cp_weight = np.asarray(cp_weight, dtype=np.float32)
    idx = np.asarray(all_indices)

    # The reference's all_indices is always the full cartesian grid; verify
    # cheaply and fall back to a host path if ever not.
    n = np.arange(S * H, dtype=idx.dtype)
    if idx.shape != (S * H, 2) or not (
        np.array_equal(idx[:, 0], n // H) and np.array_equal(idx[:, 1], n % H)
    ):
        return _np_fallback(hidden_states, idx, seq_W, hid_W, cp_weight)

    from concourse.bass_utils import run_bass_kernel_spmd

    nc = _get_program()
    in_maps = _make_in_maps(hidden_states, seq_W, hid_W, cp_weight)
    res = run_bass_kernel_spmd(nc, in_maps, list(range(N_CORES)))

    out = np.empty((B, S, H), dtype=np.float32)
    for c in range(N_CORES):
        b, half = divmod(c, 2)
        out[b, half * SH:(half + 1) * SH, :] = res.results[c]["out"]
    return out


# revision 16
# speedup vs baseline: 1.1131x; 1.0376x over previous
"""Trainium2 Bass kernel for nn_CPCircuitLayer.

Math: with all_indices the full cartesian grid (s = n // H, h = n % H),
    out[b, s, h] = sum_r seq_emb[b,s,r] * hid_emb[b,h,r] * cp[r]
                 = (seq_emb[b] @ diag(cp) @ hid_emb[b].T)[s, h]
where seq_emb[b] = X_b @ seq_W.T  (X_b = hidden_states[b], contract H)
      hid_emb[b] = X_b.T @ hid_W.T                        (contract S)

Sharding: 8 cores = (batch b, seq half) pairs. Each core holds X_b fully
(the hid factor contracts over all of S) with rows rotated so its own
seq half comes first; it PE-transposes those first 512 rows for the seq
factor, computes
    hid_embT = (hid_W*cp) @ X_b          [R, H]
    seq_embT = seq_W @ X_b[half].T       [R, S/2]
    out_half = seq_embT.T @ hid_embT     [S/2, H]
and writes its [512, 1024] slice of the output.

Matmuls run in FP32R (fp32 rounded-to-nearest at 12 mantissa bits; the
PE streams it at full rate). Inputs are pre-rounded to the FP32R bit
format on the host so the device does no conversion work. A few dummy
matmuls at kernel start warm the PE HAM clock gate before real work.
"""

import numpy as np

B, S, H, R = 4, 1024, 1024, 32
N_CORES = 8
SH = S // 2   # seq rows per core
KT = S // 128  # k-tiles over the contraction dims
MT = SH // 128  # row tiles in this core's seq half

_compiled = {}


def _np_fallback(hidden_states, all_indices, seq_W, hid_W, cp_weight):
    seq_emb = np.einsum("bsh,rh->bsr", hidden_states, seq_W)
    hid_emb = np.einsum("bsh,rs->bhr", hidden_states, hid_W)
    s_idx = all_indices[:, 0].astype(np.int64)
    h_idx = all_indices[:, 1].astype(np.int64)
    g_seq = seq_emb[:, s_idx, :]
    g_hid = hid_emb[:, h_idx, :]
    out = np.einsum("bnr,bnr,r->bn", g_seq, g_hid, cp_weight[0])
    return out.reshape(B, S, H).astype(np.float32)


def _round_f32r(a):
    """Round fp32 to the FP32R format (RNE at 12 mantissa bits), bit-exact
    with the device's own fp32->fp32r conversion."""
    b = np.ascontiguousarray(a, dtype=np.float32).view(np.uint32)
    r = (b + np.uint32(0x7FF) + ((b >> np.uint32(12)) & np.uint32(1))) \
        & np.uint32(0xFFFFF000)
    return r.view(np.float32)


def _wtile(w):
    """[K, R] -> [128, KT*R] tile layout, partition-contiguous."""
    return np.ascontiguousarray(
        w.reshape(KT, 128, R).transpose(1, 0, 2).reshape(128, KT * R))


def build_raw_program():
    import contextlib

    import concourse.bass as bass
    import concourse.mybir as mybir

    f32 = mybir.dt.float32
    f32r = mybir.dt.float32r

    nc = bass.Bass("TRN2", target_bir_lowering=False, debug=False,
                   num_devices=N_CORES, enable_partition_id=False)

    x_d = nc.dram_tensor("x", [S, H], f32r, kind="ExternalInput")
    xt_d = nc.dram_tensor("xt", [H, SH], f32r, kind="ExternalInput")
    w_d = nc.dram_tensor("w", [128, 2 * KT * R], f32r, kind="ExternalInput")
    out_d = nc.dram_tensor("out", [SH, H], f32, kind="ExternalOutput")

    NXD = 4   # x arrives in 4 DMAs of 2 row-tiles
    NTD = 4   # xt arrives in 4 DMAs of 2 row-tiles

    with contextlib.ExitStack() as _xs:
        E = _xs.enter_context
        w_t = E(nc.sbuf_tensor([128, 2 * KT * R], f32r))  # [p, sw | hw]
        x_t = E(nc.sbuf_tensor([128, KT, H], f32r))
        xt_t = E(nc.sbuf_tensor([128, KT, SH], f32r))
        hid_sb = E(nc.sbuf_tensor([R, H], f32r))
        seq_sb = E(nc.sbuf_tensor([R, SH], f32r))
        o_sb = E(nc.sbuf_tensor([128, MT, H], f32))
        hid_ps = E(nc.psum_tensor([R, H], f32))        # 2 banks
        seq_ps = E(nc.psum_tensor([R, SH], f32))       # 1 bank
        o_ps = [E(nc.psum_tensor(f"o_ps{i}", [128, 512], f32))
                for i in range(4)]                     # 4 banks
        dma_sem = E(nc.semaphore("dma_sem"))
        w_sem = E(nc.semaphore("w_sem"))
        pe_sem = E(nc.semaphore("pe_sem"))
        dve_sem = E(nc.semaphore("dve_sem"))
        act_sem = E(nc.semaphore("act_sem"))
        x_sem = [E(nc.semaphore(f"x_sem{j}")) for j in range(NXD)]
        xt_sem = [E(nc.semaphore(f"xt_sem{j}")) for j in range(NTD)]
        block = E(nc.Block(no_gpsimd_drain=True))

        sw = lambda k: w_t.ap()[:, k * R:(k + 1) * R]
        hw = lambda k: w_t.ap()[:, KT * R + k * R:KT * R + (k + 1) * R]

        # pe_sem counts (1-based): per group g in 0..3:
        #   hid(2g) n0/n1 = 6g+1, 6g+2 ; hid(2g+1) = 6g+3, 6g+4
        #   seq(2g) = 6g+5 ; seq(2g+1) = 6g+6
        # but we order: all hid groups paced by x, seq paced by xt; see below
        # dve ops: 1 = hid_c0, 2 = seq_c, 3.. = even out copies
        # act ops: 1 = hid_c1, 2.. = odd out copies

        @block.sync
        def _(sync):
            sync.dma_start(out=w_t.ap(), in_=w_d[:]).then_inc(w_sem, 16)
            for j in range(NXD):
                sync.dma_start(
                    out=x_t.ap()[:, 2 * j:2 * j + 2, :],
                    in_=x_d[j * 256:(j + 1) * 256, :].rearrange(
                        "(t p) h -> p t h", p=128),
                ).then_inc(x_sem[j], 16)
                sync.dma_start(
                    out=xt_t.ap()[:, 2 * j:2 * j + 2, :],
                    in_=xt_d[j * 256:(j + 1) * 256, :].rearrange(
                        "(t p) s -> p t s", p=128),
                ).then_inc(xt_sem[j], 16)
            # out DMAs: chunk j = (m, n) once its copy lands in o_sb
            for j in range(2 * MT):
                m, n = divmod(j, 2)
                if j % 2 == 0:
                    sync.wait_ge(dve_sem, 3 + j // 2)
                else:
                    sync.wait_ge(act_sem, 2 + (j - 1) // 2)
                sync.dma_start(
                    out=out_d[m * 128:(m + 1) * 128, n * 512:(n + 1) * 512],
                    in_=o_sb.ap()[:, m, n * 512:(n + 1) * 512],
                ).then_inc(dma_sem, 16)
            sync.wait_ge(dma_sem, 16 * 2 * MT)

        @block.tensor
        def _(tensor):
            mm_i = 0

            def hid_mm(k):
                nonlocal mm_i
                for n in range(2):
                    nc.tensor.matmul(
                        hid_ps.ap()[:, n * 512:(n + 1) * 512],
                        hw(k), x_t.ap()[:, k, n * 512:(n + 1) * 512],
                        start=(k == 0), stop=(k == KT - 1),
                    ).then_inc(pe_sem, 1)
                    mm_i += 1

            def seq_mm(k):
                nonlocal mm_i
                nc.tensor.matmul(
                    seq_ps.ap(), sw(k), xt_t.ap()[:, k, :],
                    start=(k == 0), stop=(k == KT - 1),
                ).then_inc(pe_sem, 1)
                mm_i += 1

            # warm-up on the weights tile
            tensor.wait_ge(w_sem, 16)
            for _ in range(4):
                nc.tensor.matmul(o_ps[0].ap()[0:R, :], w_t.ap()[:, 0:R],
                                 w_t.ap()[:, 0:512], start=True, stop=True)

            for g in range(NXD):
                tensor.wait_ge(x_sem[g], 16)
                hid_mm(2 * g)
                hid_mm(2 * g + 1)
                if g > 0:
                    tensor.wait_ge(xt_sem[g - 1], 16)
                    seq_mm(2 * (g - 1))
                    seq_mm(2 * (g - 1) + 1)
            tensor.wait_ge(xt_sem[NTD - 1], 16)
            seq_mm(2 * (NTD - 1))
            seq_mm(2 * (NTD - 1) + 1)
            # pe counts: hid n0 done @ 6*3+1=19? recompute:
            # g0: hid0(1,2) hid1(3,4); g1: hid2(5,6) hid3(7,8) seq0(9) seq1(10)
            # g2: hid4(11,12) hid5(13,14) seq2(15) seq3(16)
            # g3: hid6(17,18) hid7(19,20) seq4(21) seq5(22); tail: seq6(23) seq7(24)
            # hid_n0 done @19, n1 @20, seq @24; final: 25..32

            tensor.wait_ge(dve_sem, 2)   # hid_c0 + seq_c
            tensor.wait_ge(act_sem, 1)   # hid_c1
            for j in range(2 * MT):
                m, n = divmod(j, 2)
                if j >= 4:
                    # WAR on recycled PSUM bank (4-deep rotation)
                    if j % 2 == 0:
                        tensor.wait_ge(dve_sem, 3 + (j - 4) // 2)
                    else:
                        tensor.wait_ge(act_sem, 2 + (j - 4 - 1) // 2)
                nc.tensor.matmul(
                    o_ps[j % 4].ap(),
                    seq_sb.ap()[:, m * 128:(m + 1) * 128],
                    hid_sb.ap()[:, n * 512:(n + 1) * 512],
                    start=True, stop=True,
                ).then_inc(pe_sem, 1)

        @block.vector
        def _(vector):
            vector.wait_ge(pe_sem, 19)
            nc.vector.tensor_copy(
                hid_sb.ap()[:, 0:512],
                hid_ps.ap()[:, 0:512].bitcast(f32)).then_inc(dve_sem, 1)
            vector.wait_ge(pe_sem, 24)
            nc.vector.tensor_copy(
                seq_sb.ap(), seq_ps.ap().bitcast(f32)).then_inc(dve_sem, 1)
            for j in range(0, 2 * MT, 2):   # even out copies
                m, n = divmod(j, 2)
                vector.wait_ge(pe_sem, 24 + j + 1)
                nc.vector.tensor_copy(
                    o_sb.ap()[:, m, n * 512:(n + 1) * 512],
                    o_ps[j % 4].ap(),
                ).then_inc(dve_sem, 1)

        @block.scalar
        def _(scalar):
            scalar.wait_ge(pe_sem, 20)
            nc.scalar.copy(
                hid_sb.ap()[:, 512:1024],
                hid_ps.ap()[:, 512:1024].bitcast(f32)).then_inc(act_sem, 1)
            for j in range(1, 2 * MT, 2):   # odd out copies
                m, n = divmod(j, 2)
                scalar.wait_ge(pe_sem, 24 + j + 1)
                nc.scalar.copy(
                    o_sb.ap()[:, m, n * 512:(n + 1) * 512],
                    o_ps[j % 4].ap(),
                ).then_inc(act_sem, 1)

    return nc


def _get_program():
    if "nc" not in _compiled:
        _compiled["nc"] = build_raw_program()
    return _compiled["nc"]


def _make_in_maps(hidden_states, seq_W, hid_W, cp_weight):
    swT = _wtile(np.ascontiguousarray(seq_W.T))                    # [128, 256]
    hwT_rows = np.ascontiguousarray((hid_W * cp_weight[0][:, None]).T)  # [S, R]
    # per-half row rotation: own seq half first (hid contraction over S is
    # order-invariant as long as x rows and hw rows permute together)
    w_rot = [
        _round_f32r(np.concatenate([swT, _wtile(np.concatenate(
            [hwT_rows[half * SH:], hwT_rows[:half * SH]], axis=0))], axis=1))
        for half in range(2)
    ]
    in_maps = []
    for c in range(N_CORES):
        b, half = divmod(c, 2)
        xb = _round_f32r(hidden_states[b])
        if half:
            xb = np.ascontiguousarray(
                np.concatenate([xb[SH:], xb[:SH]], axis=0))
        in_maps.append({
            "x": xb,
            "xt": np.ascontiguousarray(xb[:SH, :].T),
            "w": w_rot[half],
        })
    return in_maps


def kernel(hidden_states, all_indices, seq_W, hid_W, cp_weight):
    hidden_states = np.asarray(hidden_states, dtype=np.float32)
    seq_W = np.asarray(seq_W, dtype=np.float32)
    hid_W = np.asarray(hid_W, dtype=np.float32)
    cp_weight = np.asarray(cp_weight, dtype=np.float32)
    idx = np.asarray(all_indices)

    # The reference's all_indices is always the full cartesian grid; verify
    # cheaply and fall back to a host path if ever not.
    n = np.arange(S * H, dtype=idx.dtype)
    if idx.shape != (S * H, 2) or not (
        np.array_equal(idx[:, 0], n // H) and np.array_equal(idx[:, 1], n % H)
    ):
        return _np_fallback(hidden_states, idx, seq_W, hid_W, cp_weight)

    from concourse.bass_utils import run_bass_kernel_spmd

    nc = _get_program()
    in_maps = _make_in_maps(hidden_states, seq_W, hid_W, cp_weight)
    res = run_bass_kernel_spmd(nc, in_maps, list(range(N_CORES)))

    out = np.empty((B, S, H), dtype=np.float32)
    for c in range(N_CORES):
        b, half = divmod(c, 2)
        out[b, half * SH:(half + 1) * SH, :] = res.results[c]["out"]
    return out
